# revision 23
# baseline (speedup 1.0000x reference)
"""L2 (spectral) contrastive loss on 8 Trainium2 NeuronCores.

Math: with G_x = x.T @ x and G_y = y.T @ y (both [D, D]),
    sum_{i,j} <x_i, y_j>^2 = ||x @ y.T||_F^2 = tr(G_x @ G_y) = sum(G_x * G_y)
so the loss needs only the two Gram matrices (2*N*D^2 MACs) instead of the
[N, N] pairwise product (N^2*D MACs).

Split-collective structure (vs the single 1.38MB fp16 AllReduce baseline):
  - Rows are split across the 8 cores. Both Gram triangles feed ONE fp16
    ReduceScatter laid out k-block-outer [8, 2, 128, 336], so core c
    receives the summed (Gy, Gx) column-block pair c as [2, 128, 336] -
    full 128 partitions, plain-slice readback, and the chunk dot is a
    single [128, 336] fp16 STT (the per-chunk partial dots sum across
    cores to the full sum(Gx_tot * Gy_tot)). An fp8e4 payload returns
    garbage from the collectives firmware; fp16/fp32 work.
  - The loss is linear in per-core scalars: the PE partition-reduce uses a
    weight vector of inv_nn1/SCALE and the z columns are pre-scaled/
    negated to compensate, so l_c = reduce(pfin) and the scalar AllReduce
    OUTPUT already is the loss - post-collective work is one 4-byte DMA.
  - Inputs stream on 3 DMA queues (y chunks first), fp8 casts on the
    scalar engine (gpsimd casts were 6x slower and the old critical path),
    G_y computed kk-outer with all 6 triangle slabs resident in PSUM
    (exactly 8 banks) so the Gram finishes with the last y cast.
  - z_i = <x_i, y_i> on the vector engine from fp32; z terms ride the
    scalar AllReduce in exact f32.

Measured structure (ntff profile): the collectives firmware imposes a
fixed latency floor - CC cores boot ~20µs into the NEFF, process a
runtime-inserted BARRIER for 26-44µs (run-to-run variance, likely
dispatch skew), and pick up the first data collective 11.2µs after the
barrier ends - so the ReduceScatter cannot start before ~60-77µs no
matter when it's triggered (trigger is ~45-50µs). Collective cost =
~10µs/op fixed + ~11-18µs/MB, which makes AllReduce(Gy)+full-dot,
ReduceScatter(both)+chunk-dot, and single-AllReduce(both) all equal
within ~1µs; compute is never the critical path.
  (A 3-round remote-DMA butterfly would bypass the firmware but
  intermittently wedges the device - keep the firmware collectives.)
"""
import numpy as np
from contextlib import ExitStack

from concourse import bacc, tile, mybir
from concourse.bass_utils import run_bass_kernel_spmd

N_CORES = 8
N, D = 8192, 768
ROWS = N // N_CORES          # 1024 rows per core
P = 128                      # SBUF partitions
KCH = ROWS // P              # 8 contraction chunks per core
KK = KCH // 2                # 4 DoubleRow steps (2 chunks per pass)
MS = D // P                  # 6 output slabs per Gram

WIDTHS = [D - P * m for m in range(MS)]              # [768,640,512,384,256,128]
COFF = [sum(WIDTHS[:m]) for m in range(MS)]          # prefix offsets
GCOLS = sum(WIDTHS)                                  # 2688 triangle cols

F32 = mybir.dt.float32
F16 = mybir.dt.float16
FP8 = mybir.dt.float8e4

# Gx pre-scale: keeps fp16 dot products (diag ~8192 * 8192) in fp16 range
SCALE = 2.0 ** -13

_CACHE = {}


def _mm_chunks(width):
    """Split [0, width) at the 512-column PSUM bank boundary."""
    if width <= 512:
        return [(0, width)]
    return [(0, 512), (512, width)]


def _build():
    nc = bacc.Bacc("TRN2", target_bir_lowering=False, debug=False,
                   num_devices=N_CORES)
    x_ap = nc.dram_tensor("x", [ROWS, D], F32, kind="ExternalInput").ap()
    y_ap = nc.dram_tensor("y", [ROWS, D], F32, kind="ExternalInput").ap()
    loss_ap = nc.dram_tensor("loss", [1, 1], F32, kind="ExternalOutput").ap()

    inv_nn1 = 1.0 / (float(N) * (N - 1))

    with tile.TileContext(nc) as tc:
        with ExitStack() as ctx:
            sb = ctx.enter_context(tc.tile_pool(name="sb", bufs=1))
            ps = ctx.enter_context(tc.tile_pool(name="ps", bufs=1, space="PSUM"))
            dram = ctx.enter_context(tc.tile_pool(name="dram", bufs=1,
                                                  space="DRAM"))

            # ---- load inputs: [1024, 768] -> [128p, 8k, 768] on 4 DMA
            # queues, ALL y chunks issued before any x so G_y (the only
            # Gram that rides the big collective) completes first ----
            xt = sb.tile([P, KCH, D], F32)
            yt = sb.tile([P, KCH, D], F32)
            xr = x_ap.rearrange("(n p) d -> p n d", p=P)
            yr = y_ap.rearrange("(n p) d -> p n d", p=P)
            qeng = (nc.sync, nc.scalar, nc.gpsimd)
            for k in range(KCH):
                qeng[k % 3].dma_start(yt[:, k, :], yr[:, k, :])
            for k in range(KCH):
                qeng[k % 3].dma_start(xt[:, k, :], xr[:, k, :])

            # ---- fp8 casts, all on the scalar engine (~0.7µs each; the
            # old gpsimd path was 3.2µs each and the critical path) ----
            yb = sb.tile([P, KCH, D], FP8)
            xb = sb.tile([P, KCH, D], FP8)
            for k in range(KCH):
                nc.scalar.copy(yb[:, k, :], yt[:, k, :])
            for k in range(KCH):
                nc.scalar.copy(xb[:, k, :], xt[:, k, :])

            # partition-reduce weights: ones scaled by inv_nn1/SCALE so the
            # PE reduce also applies the dot normalization; the z columns
            # are pre-scaled to compensate (see stage cols 3-4)
            C0 = inv_nn1 / SCALE
            wvec = sb.tile([P, 1], F32)
            nc.vector.memset(wvec[:], C0)

            # ---- G_y: kk-outer with all 6 triangle slabs resident in
            # PSUM (2+2+1+1+1+1 = exactly 8 banks), consuming each cast
            # chunk pair as it arrives ----
            slabs = []
            for m in range(MS):
                slab = ps.tile([P, WIDTHS[m]], F32, tag=f"slab{m}", bufs=1,
                               name=f"slab{m}")
                slabs.append(slab)

            pack = sb.tile([P, GCOLS], F16)       # G_y fp16 AR payload
            gxp = sb.tile([P, GCOLS], F16)        # local Gx, scaled, x2 off-diag

            def gram_pass(src):
                for kk in range(KK):
                    for m in range(MS):
                        for (c0, c1) in _mm_chunks(WIDTHS[m]):
                            nc.tensor.matmul(
                                slabs[m][:, c0:c1],
                                src[:, 2 * kk:2 * kk + 2, P * m:P * (m + 1)],
                                src[:, 2 * kk:2 * kk + 2,
                                    P * m + c0:P * m + c1],
                                start=(kk == 0),
                                stop=(kk == KK - 1),
                                perf_mode=mybir.MatmulPerfMode.DoubleRow,
                                skip_group_check=True,
                            )

            gram_pass(yb)

            # ---- pack G_y slabs to fp16 (vector); both packs feed one
            # ReduceScatter. cin is laid out k-block-outer [8, 2, 128, 336]
            # so RS chunk c is exactly [2, 128, 336]: the (Gy, Gx) column
            # block pair c with the FULL 128 partitions - the readback is
            # two plain slices and the chunk dot keeps all DVE lanes ----
            CW = GCOLS // N_CORES                 # 336 cols per block
            cin = dram.tile([N_CORES, 2, P, CW], F16)
            cout = dram.tile([2, P, CW], F16)

            def stage_pack(src_sb, plane, k0, k1, eng):
                dst = cin[k0:k1, plane, :, :].rearrange("k p j -> p k j")
                src = src_sb[:, k0 * CW:k1 * CW].rearrange(
                    "p (k j) -> p k j", j=CW)
                eng.dma_start(dst, src)

            for m in range(MS):
                off, w = COFF[m], WIDTHS[m]
                nc.vector.tensor_copy(pack[:, off:off + w], slabs[m][:, 0:w])
            # 336-aligned staging pieces: cols 0:1344 ready after slab 1
            # (covers to col 1408), 1344:2688 after slab 5
            stage_pack(pack, 0, 0, 4, nc.gpsimd)
            stage_pack(pack, 0, 4, N_CORES, nc.gpsimd)

            # ---- G_x into the same PSUM slabs (WAR on the pack copies);
            # packed with the 2^-13 pre-scale and the x2 weight on
            # strict-right (off-diagonal) columns, staged next to G_y ----
            gram_pass(xb)
            for m in range(MS):
                off, w = COFF[m], WIDTHS[m]
                nc.scalar.mul(gxp[:, off:off + P], slabs[m][:, 0:P], SCALE)
                if w > P:
                    nc.scalar.mul(gxp[:, off + P:off + w],
                                  slabs[m][:, P:w], 2.0 * SCALE)
            stage_pack(gxp, 1, 0, 4, nc.sync)
            stage_pack(gxp, 1, 4, N_CORES, nc.gpsimd)

            nc.gpsimd.collective_compute(
                "ReduceScatter",
                mybir.AluOpType.add,
                replica_groups=[list(range(N_CORES))],
                ins=[cin.opt()],
                outs=[cout.opt()],
            )

            # ---- diagonal terms z_i = <x_i, y_i> from fp32 (vector) ----
            zscr = sb.tile([P, D], F32)
            zcols = sb.tile([P, KCH], F32)
            for k in range(KCH):
                nc.vector.scalar_tensor_tensor(
                    zscr[:], xt[:, k, :], 1.0, yt[:, k, :],
                    mybir.AluOpType.mult, mybir.AluOpType.mult,
                    accum_out=zcols[:, k:k + 1],
                )
            zsq = sb.tile([P, KCH], F32)
            stage = sb.tile([P, 3], F32)
            # stage cols 1-2 carry the z terms pre-divided by C0 (and
            # negated, since they're subtracted) so that C0 * sum_p(stage)
            # via the PE reduce yields the finished per-core contribution:
            #   col 1 -> -(2/N)*sum z,  col 2 -> -inv_nn1*sum z^2
            zred = sb.tile([P, 2], F32)
            nc.vector.tensor_reduce(zred[:, 0:1], zcols[:],
                                    mybir.AxisListType.X, mybir.AluOpType.add)
            nc.vector.scalar_tensor_tensor(
                zsq[:], zcols[:], 1.0, zcols[:],
                mybir.AluOpType.mult, mybir.AluOpType.mult,
                accum_out=zred[:, 1:2],
            )
            nc.vector.tensor_scalar_mul(stage[:, 1:2], zred[:, 0:1],
                                        -2.0 * SCALE * (N - 1))
            nc.vector.tensor_scalar_mul(stage[:, 2:3], zred[:, 1:2], -SCALE)

            # ---- after the ReduceScatter: read back this core's (Gy, Gx)
            # summed column-block pair, one fp16 STT dots them ----
            ga = sb.tile([P, CW], F16)
            gb = sb.tile([P, CW], F16)
            dscr = sb.tile([P, CW], F16)
            nc.sync.dma_start(ga[:], cout[0, :, :])
            nc.scalar.dma_start(gb[:], cout[1, :, :])
            nc.vector.scalar_tensor_tensor(
                dscr[:], ga[:], 1.0, gb[:],
                mybir.AluOpType.mult, mybir.AluOpType.mult,
                accum_out=stage[:, 0:1],
            )

            # ---- partition reduction via PE (wvec^T @ stage) applies the
            # C0 normalization; a single horizontal reduce then yields the
            # finished per-core loss contribution, so the scalar AllReduce
            # output IS the loss ----
            pfin = slabs[5][0:1, 0:3]
            nc.tensor.matmul(pfin, wvec[:, 0:1], stage[:, 0:3],
                             start=True, stop=True)
            lc = sb.tile([1, 1], F32)
            nc.vector.tensor_reduce(lc[:], pfin,
                                    mybir.AxisListType.X, mybir.AluOpType.add)

            cin2 = dram.tile([1, 1], F32)
            cout2 = dram.tile([1, 1], F32, addr_space="Shared")
            nc.sync.dma_start(cin2[:], lc[:])
            nc.gpsimd.collective_compute(
                "AllReduce",
                mybir.AluOpType.add,
                replica_groups=[list(range(N_CORES))],
                ins=[cin2.opt()],
                outs=[cout2.opt()],
            )
            nc.sync.dma_start(loss_ap[:], cout2[:])

    nc.compile()
    return nc


def _get_nc():
    if "nc" not in _CACHE:
        _CACHE["nc"] = _build()
    return _CACHE["nc"]


def _run(x, y, trace=False, **trace_kwargs):
    nc = _get_nc()
    x = np.ascontiguousarray(np.asarray(x, dtype=np.float32))
    y = np.ascontiguousarray(np.asarray(y, dtype=np.float32))
    assert x.shape == (N, D) and y.shape == (N, D)
    in_maps = [
        {"x": x[c * ROWS:(c + 1) * ROWS], "y": y[c * ROWS:(c + 1) * ROWS]}
        for c in range(N_CORES)
    ]
    res = run_bass_kernel_spmd(nc, in_maps, list(range(N_CORES)), trace=trace,
                               **trace_kwargs)
    loss = np.float32(res.results[0]["loss"][0, 0])
    return np.asarray(loss, dtype=np.float32).reshape(()), res


def kernel(x, y):
    out, _ = _run(x, y, trace=False)
    return out


# revision 28
# speedup vs baseline: 1.0802x; 1.0802x over previous
"""L2 (spectral) contrastive loss on 8 Trainium2 NeuronCores.

Math: with G_x = x.T @ x and G_y = y.T @ y (both [D, D]),
    sum_{i,j} <x_i, y_j>^2 = ||x @ y.T||_F^2 = tr(G_x @ G_y) = sum(G_x * G_y)
so the loss needs only the two Gram matrices (2*N*D^2 MACs) instead of the
[N, N] pairwise product (N^2*D MACs).

Split-collective structure (vs the single 1.38MB fp16 AllReduce baseline):
  - Rows are split across the 8 cores. Both Gram triangles feed ONE fp16
    ReduceScatter laid out k-block-outer [8, 2, 128, 336], so core c
    receives the summed (Gy, Gx) column-block pair c as [2, 128, 336] -
    full 128 partitions, plain-slice readback, and the chunk dot is a
    single [128, 336] fp16 STT (the per-chunk partial dots sum across
    cores to the full sum(Gx_tot * Gy_tot)). An fp8e4 payload returns
    garbage from the collectives firmware; fp16/fp32 work.
  - The loss is linear in per-core scalars: the PE partition-reduce uses a
    weight vector of inv_nn1/SCALE and the z columns are pre-scaled/
    negated to compensate, so l_c = reduce(pfin) and the scalar AllReduce
    OUTPUT already is the loss - post-collective work is one 4-byte DMA.
  - Inputs stream on 3 DMA queues (y chunks first), fp8 casts on the
    scalar engine (gpsimd casts were 6x slower and the old critical path),
    G_y computed kk-outer with all 6 triangle slabs resident in PSUM
    (exactly 8 banks) so the Gram finishes with the last y cast.
  - z_i = <x_i, y_i> on the vector engine from fp32; z terms ride the
    scalar AllReduce in exact f32.

Measured structure (ntff profile): the collectives firmware imposes a
fixed latency floor - CC cores boot ~20µs into the NEFF, process a
runtime-inserted BARRIER for 26-44µs (run-to-run variance, likely
dispatch skew), and pick up the first data collective 11.2µs after the
barrier ends - so the ReduceScatter cannot start before ~60-77µs no
matter when it's triggered (trigger is ~45-50µs). Collective cost =
~10µs/op fixed + ~11-18µs/MB, which makes AllReduce(Gy)+full-dot,
ReduceScatter(both)+chunk-dot, and single-AllReduce(both) all equal
within ~1µs; compute is never the critical path.
  (A 3-round remote-DMA butterfly would bypass the firmware but
  intermittently wedges the device - keep the firmware collectives.)
"""
import numpy as np
from contextlib import ExitStack

from concourse import bacc, tile, mybir
from concourse.bass_utils import run_bass_kernel_spmd

N_CORES = 8
N, D = 8192, 768
ROWS = N // N_CORES          # 1024 rows per core
P = 128                      # SBUF partitions
KCH = ROWS // P              # 8 contraction chunks per core
KK = KCH // 2                # 4 DoubleRow steps (2 chunks per pass)
MS = D // P                  # 6 output slabs per Gram

WIDTHS = [D - P * m for m in range(MS)]              # [768,640,512,384,256,128]
COFF = [sum(WIDTHS[:m]) for m in range(MS)]          # prefix offsets
GCOLS = sum(WIDTHS)                                  # 2688 triangle cols

F32 = mybir.dt.float32
F16 = mybir.dt.float16
FP8 = mybir.dt.float8e4

# Gx pre-scale: keeps fp16 dot products (diag ~8192 * 8192) in fp16 range
SCALE = 2.0 ** -13

_CACHE = {}


def _mm_chunks(width):
    """Split [0, width) at the 512-column PSUM bank boundary."""
    if width <= 512:
        return [(0, width)]
    return [(0, 512), (512, width)]


def _build():
    nc = bacc.Bacc("TRN2", target_bir_lowering=False, debug=False,
                   num_devices=N_CORES)
    x_ap = nc.dram_tensor("x", [ROWS, D], F32, kind="ExternalInput").ap()
    y_ap = nc.dram_tensor("y", [ROWS, D], F32, kind="ExternalInput").ap()
    loss_ap = nc.dram_tensor("loss", [1, 1], F32, kind="ExternalOutput").ap()

    inv_nn1 = 1.0 / (float(N) * (N - 1))

    with tile.TileContext(nc) as tc:
        with ExitStack() as ctx:
            sb = ctx.enter_context(tc.tile_pool(name="sb", bufs=1))
            ps = ctx.enter_context(tc.tile_pool(name="ps", bufs=1, space="PSUM"))
            dram = ctx.enter_context(tc.tile_pool(name="dram", bufs=1,
                                                  space="DRAM"))

            # ---- load inputs: [1024, 768] -> [128p, 8k, 768] on 4 DMA
            # queues, ALL y chunks issued before any x so G_y (the only
            # Gram that rides the big collective) completes first ----
            xt = sb.tile([P, KCH, D], F32)
            yt = sb.tile([P, KCH, D], F32)
            xr = x_ap.rearrange("(n p) d -> p n d", p=P)
            yr = y_ap.rearrange("(n p) d -> p n d", p=P)
            qeng = (nc.sync, nc.scalar, nc.gpsimd)
            for k in range(KCH):
                qeng[k % 3].dma_start(yt[:, k, :], yr[:, k, :])
            for k in range(KCH):
                qeng[k % 3].dma_start(xt[:, k, :], xr[:, k, :])

            # ---- fp8 casts: y and the first x chunks on scalar (~0.9µs
            # each; gpsimd was 3.2µs), the x tail on vector after its Gy
            # packs - so the last cast tracks the last DMA arrival and the
            # G_x Gram (which gates the ReduceScatter trigger) ends early ----
            yb = sb.tile([P, KCH, D], FP8)
            xb = sb.tile([P, KCH, D], FP8)
            for k in range(KCH):
                nc.scalar.copy(yb[:, k, :], yt[:, k, :])
            for k in range(4):
                nc.scalar.copy(xb[:, k, :], xt[:, k, :])

            # partition-reduce weights: ones scaled by inv_nn1/SCALE so the
            # PE reduce also applies the dot normalization; the z columns
            # are pre-scaled to compensate (see stage cols 3-4)
            C0 = inv_nn1 / SCALE
            wvec = sb.tile([P, 1], F32)
            nc.vector.memset(wvec[:], C0)

            # ---- G_y: kk-outer with all 6 triangle slabs resident in
            # PSUM (2+2+1+1+1+1 = exactly 8 banks), consuming each cast
            # chunk pair as it arrives ----
            slabs = []
            for m in range(MS):
                slab = ps.tile([P, WIDTHS[m]], F32, tag=f"slab{m}", bufs=1,
                               name=f"slab{m}")
                slabs.append(slab)

            pack = sb.tile([P, GCOLS], F16)       # G_y fp16 AR payload
            gxp = sb.tile([P, GCOLS], F16)        # local Gx, scaled, x2 off-diag

            def gram_pass(src):
                for kk in range(KK):
                    for m in range(MS):
                        for (c0, c1) in _mm_chunks(WIDTHS[m]):
                            nc.tensor.matmul(
                                slabs[m][:, c0:c1],
                                src[:, 2 * kk:2 * kk + 2, P * m:P * (m + 1)],
                                src[:, 2 * kk:2 * kk + 2,
                                    P * m + c0:P * m + c1],
                                start=(kk == 0),
                                stop=(kk == KK - 1),
                                perf_mode=mybir.MatmulPerfMode.DoubleRow,
                                skip_group_check=True,
                            )

            gram_pass(yb)

            # ---- pack G_y slabs to fp16 (vector); both packs feed one
            # ReduceScatter. cin is laid out k-block-outer [8, 2, 128, 336]
            # so RS chunk c is exactly [2, 128, 336]: the (Gy, Gx) column
            # block pair c with the FULL 128 partitions - the readback is
            # two plain slices and the chunk dot keeps all DVE lanes ----
            CW = GCOLS // N_CORES                 # 336 cols per block
            cin = dram.tile([N_CORES, 2, P, CW], F16)
            cout = dram.tile([2, P, CW], F16)

            def stage_pack(src_sb, plane, k0, k1, eng):
                dst = cin[k0:k1, plane, :, :].rearrange("k p j -> p k j")
                src = src_sb[:, k0 * CW:k1 * CW].rearrange(
                    "p (k j) -> p k j", j=CW)
                eng.dma_start(dst, src)

            for m in range(MS):
                off, w = COFF[m], WIDTHS[m]
                nc.vector.tensor_copy(pack[:, off:off + w], slabs[m][:, 0:w])
            # 336-aligned staging pieces: cols 0:1344 ready after slab 1
            # (covers to col 1408), 1344:2688 after slab 5
            stage_pack(pack, 0, 0, 4, nc.gpsimd)
            stage_pack(pack, 0, 4, N_CORES, nc.gpsimd)
            # x-cast tail on vector, right after the Gy packs
            for k in range(4, KCH):
                nc.vector.tensor_copy(xb[:, k, :], xt[:, k, :])

            # ---- G_x into the same PSUM slabs (WAR on the pack copies);
            # packed with the 2^-13 pre-scale and the x2 weight on
            # strict-right (off-diagonal) columns, staged next to G_y ----
            gram_pass(xb)
            for m in range(MS):
                off, w = COFF[m], WIDTHS[m]
                if m % 2 == 0:
                    nc.scalar.mul(gxp[:, off:off + P], slabs[m][:, 0:P],
                                  SCALE)
                    if w > P:
                        nc.scalar.mul(gxp[:, off + P:off + w],
                                      slabs[m][:, P:w], 2.0 * SCALE)
                else:
                    nc.vector.tensor_scalar_mul(gxp[:, off:off + P],
                                                slabs[m][:, 0:P], SCALE)
                    if w > P:
                        nc.vector.tensor_scalar_mul(
                            gxp[:, off + P:off + w],
                            slabs[m][:, P:w], 2.0 * SCALE)
            stage_pack(gxp, 1, 0, 4, nc.sync)
            stage_pack(gxp, 1, 4, N_CORES, nc.gpsimd)

            nc.gpsimd.collective_compute(
                "ReduceScatter",
                mybir.AluOpType.add,
                replica_groups=[list(range(N_CORES))],
                ins=[cin.opt()],
                outs=[cout.opt()],
            )

            # ---- diagonal terms z_i = <x_i, y_i> from fp32 (vector) ----
            zscr = sb.tile([P, D], F32)
            zcols = sb.tile([P, KCH], F32)
            for k in range(KCH):
                nc.vector.scalar_tensor_tensor(
                    zscr[:], xt[:, k, :], 1.0, yt[:, k, :],
                    mybir.AluOpType.mult, mybir.AluOpType.mult,
                    accum_out=zcols[:, k:k + 1],
                )
            zsq = sb.tile([P, KCH], F32)
            stage = sb.tile([P, 3], F32)
            # stage cols 1-2 carry the z terms pre-divided by C0 (and
            # negated, since they're subtracted) so that C0 * sum_p(stage)
            # via the PE reduce yields the finished per-core contribution:
            #   col 1 -> -(2/N)*sum z,  col 2 -> -inv_nn1*sum z^2
            zred = sb.tile([P, 2], F32)
            nc.vector.tensor_reduce(zred[:, 0:1], zcols[:],
                                    mybir.AxisListType.X, mybir.AluOpType.add)
            nc.vector.scalar_tensor_tensor(
                zsq[:], zcols[:], 1.0, zcols[:],
                mybir.AluOpType.mult, mybir.AluOpType.mult,
                accum_out=zred[:, 1:2],
            )
            nc.vector.tensor_scalar_mul(stage[:, 1:2], zred[:, 0:1],
                                        -2.0 * SCALE * (N - 1))
            nc.vector.tensor_scalar_mul(stage[:, 2:3], zred[:, 1:2], -SCALE)

            # ---- after the ReduceScatter: read back this core's (Gy, Gx)
            # summed column-block pair in ONE DMA (one completion sem),
            # one fp16 STT dots them ----
            gboth = sb.tile([P, 2 * CW], F16)
            dscr = sb.tile([P, CW], F16)
            nc.sync.dma_start(
                gboth.rearrange("p (two j) -> p two j", two=2),
                cout.rearrange("two p j -> p two j"),
            )
            nc.vector.scalar_tensor_tensor(
                dscr[:], gboth[:, 0:CW], 1.0, gboth[:, CW:2 * CW],
                mybir.AluOpType.mult, mybir.AluOpType.mult,
                accum_out=stage[:, 0:1],
            )

            # ---- partition reduction via PE (wvec^T @ stage) applies the
            # C0 normalization; a single horizontal reduce then yields the
            # finished per-core loss contribution, so the scalar AllReduce
            # output IS the loss ----
            pfin = slabs[5][0:1, 0:3]
            nc.tensor.matmul(pfin, wvec[:, 0:1], stage[:, 0:3],
                             start=True, stop=True)
            lc = sb.tile([1, 1], F32)
            nc.vector.tensor_reduce(lc[:], pfin,
                                    mybir.AxisListType.X, mybir.AluOpType.add)

            cin2 = dram.tile([1, 1], F32)
            cout2 = dram.tile([1, 1], F32, addr_space="Shared")
            nc.sync.dma_start(cin2[:], lc[:])
            nc.gpsimd.collective_compute(
                "AllReduce",
                mybir.AluOpType.add,
                replica_groups=[list(range(N_CORES))],
                ins=[cin2.opt()],
                outs=[cout2.opt()],
            )
            nc.sync.dma_start(loss_ap[:], cout2[:])

    nc.compile()
    return nc


def _get_nc():
    if "nc" not in _CACHE:
        _CACHE["nc"] = _build()
    return _CACHE["nc"]


def _run(x, y, trace=False, **trace_kwargs):
    nc = _get_nc()
    x = np.ascontiguousarray(np.asarray(x, dtype=np.float32))
    y = np.ascontiguousarray(np.asarray(y, dtype=np.float32))
    assert x.shape == (N, D) and y.shape == (N, D)
    in_maps = [
        {"x": x[c * ROWS:(c + 1) * ROWS], "y": y[c * ROWS:(c + 1) * ROWS]}
        for c in range(N_CORES)
    ]
    res = run_bass_kernel_spmd(nc, in_maps, list(range(N_CORES)), trace=trace,
                               **trace_kwargs)
    loss = np.float32(res.results[0]["loss"][0, 0])
    return np.asarray(loss, dtype=np.float32).reshape(()), res


def kernel(x, y):
    out, _ = _run(x, y, trace=False)
    return out


# revision 34
# speedup vs baseline: 1.1813x; 1.0936x over previous
"""L2 (spectral) contrastive loss on 8 Trainium2 NeuronCores.

Math: with G_x = x.T @ x and G_y = y.T @ y (both [D, D]),
    sum_{i,j} <x_i, y_j>^2 = ||x @ y.T||_F^2 = tr(G_x @ G_y) = sum(G_x * G_y)
so the loss needs only the two Gram matrices (2*N*D^2 MACs) instead of the
[N, N] pairwise product (N^2*D MACs).

Split-collective structure (vs the single 1.38MB fp16 AllReduce baseline):
  - Rows are split across the 8 cores. Both Gram triangles feed ONE fp16
    ReduceScatter laid out k-block-outer [8, 2, 128, 336], so core c
    receives the summed (Gy, Gx) column-block pair c as [2, 128, 336] -
    full 128 partitions, plain-slice readback, and the chunk dot is a
    single [128, 336] fp16 STT (the per-chunk partial dots sum across
    cores to the full sum(Gx_tot * Gy_tot)). An fp8e4 payload returns
    garbage from the collectives firmware; fp16/fp32 work.
  - The loss is linear in per-core scalars: the PE partition-reduce uses a
    weight vector of inv_nn1/SCALE and the z columns are pre-scaled/
    negated to compensate, so l_c = reduce(pfin) and the scalar AllReduce
    OUTPUT already is the loss - post-collective work is one 4-byte DMA.
  - Inputs stream on 3 DMA queues (y chunks first), fp8 casts on the
    scalar engine (gpsimd casts were 6x slower and the old critical path),
    G_y computed kk-outer with all 6 triangle slabs resident in PSUM
    (exactly 8 banks) so the Gram finishes with the last y cast.
  - z_i = <x_i, y_i> on the vector engine from fp32; z terms ride the
    scalar AllReduce in exact f32.

Measured structure (ntff profile): the collectives firmware imposes a
fixed latency floor - CC cores boot ~20µs into the NEFF, process a
runtime-inserted BARRIER for 26-53µs (run-to-run dispatch-skew variance),
and pick up the first data collective 11.2µs after the barrier ends -
so the ReduceScatter cannot start before ~60-85µs no matter when it's
triggered. Collective cost ~10µs/op fixed + ~11-18µs/MB makes all
topologies (AR(Gy)+full dot, RS(both)+chunk dot, single big AR) equal
within ~1µs; compute is never the critical path. Late triggers (>~48µs)
measurably stretch the collective - keep staging off the slow gpsimd
SW-DGE queue if changing this code.
  (A 3-round remote-DMA butterfly would bypass the firmware but
  intermittently wedges the device - keep the firmware collectives.)
"""
import numpy as np
from contextlib import ExitStack

from concourse import bacc, tile, mybir
from concourse.bass_utils import run_bass_kernel_spmd

N_CORES = 8
N, D = 8192, 768
ROWS = N // N_CORES          # 1024 rows per core
P = 128                      # SBUF partitions
KCH = ROWS // P              # 8 contraction chunks per core
KK = KCH // 2                # 4 DoubleRow steps (2 chunks per pass)
MS = D // P                  # 6 output slabs per Gram

WIDTHS = [D - P * m for m in range(MS)]              # [768,640,512,384,256,128]
COFF = [sum(WIDTHS[:m]) for m in range(MS)]          # prefix offsets
GCOLS = sum(WIDTHS)                                  # 2688 triangle cols

F32 = mybir.dt.float32
F16 = mybir.dt.float16
FP8 = mybir.dt.float8e4

# Gx pre-scale: keeps fp16 dot products (diag ~8192 * 8192) in fp16 range
SCALE = 2.0 ** -13

_CACHE = {}


def _mm_chunks(width):
    """Split [0, width) at the 512-column PSUM bank boundary."""
    if width <= 512:
        return [(0, width)]
    return [(0, 512), (512, width)]


def _build():
    nc = bacc.Bacc("TRN2", target_bir_lowering=False, debug=False,
                   num_devices=N_CORES)
    x_ap = nc.dram_tensor("x", [ROWS, D], F32, kind="ExternalInput").ap()
    y_ap = nc.dram_tensor("y", [ROWS, D], F32, kind="ExternalInput").ap()
    loss_ap = nc.dram_tensor("loss", [1, 1], F32, kind="ExternalOutput").ap()

    inv_nn1 = 1.0 / (float(N) * (N - 1))

    with tile.TileContext(nc) as tc:
        with ExitStack() as ctx:
            sb = ctx.enter_context(tc.tile_pool(name="sb", bufs=1))
            ps = ctx.enter_context(tc.tile_pool(name="ps", bufs=1, space="PSUM"))
            dram = ctx.enter_context(tc.tile_pool(name="dram", bufs=1,
                                                  space="DRAM"))

            # ---- load inputs: [1024, 768] -> [128p, 8k, 768] on 4 DMA
            # queues, ALL y chunks issued before any x so G_y (the only
            # Gram that rides the big collective) completes first ----
            xt = sb.tile([P, KCH, D], F32)
            yt = sb.tile([P, KCH, D], F32)
            xr = x_ap.rearrange("(n p) d -> p n d", p=P)
            yr = y_ap.rearrange("(n p) d -> p n d", p=P)
            qeng = (nc.sync, nc.scalar, nc.gpsimd)
            for k in range(KCH):
                qeng[k % 3].dma_start(yt[:, k, :], yr[:, k, :])
            for k in range(KCH):
                qeng[k % 3].dma_start(xt[:, k, :], xr[:, k, :])

            # ---- fp8 casts, all on the scalar engine (~0.7µs each; the
            # old gpsimd path was 3.2µs each and the critical path) ----
            yb = sb.tile([P, KCH, D], FP8)
            xb = sb.tile([P, KCH, D], FP8)
            for k in range(KCH):
                nc.scalar.copy(yb[:, k, :], yt[:, k, :])
            for k in range(KCH):
                nc.scalar.copy(xb[:, k, :], xt[:, k, :])

            # partition-reduce weights: ones scaled by inv_nn1/SCALE so the
            # PE reduce also applies the dot normalization; the z columns
            # are pre-scaled to compensate (see stage cols 3-4)
            C0 = inv_nn1 / SCALE
            wvec = sb.tile([P, 1], F32)
            nc.vector.memset(wvec[:], C0)

            # ---- G_y: kk-outer with all 6 triangle slabs resident in
            # PSUM (2+2+1+1+1+1 = exactly 8 banks), consuming each cast
            # chunk pair as it arrives ----
            slabs = []
            for m in range(MS):
                slab = ps.tile([P, WIDTHS[m]], F32, tag=f"slab{m}", bufs=1,
                               name=f"slab{m}")
                slabs.append(slab)

            pack = sb.tile([P, GCOLS], F16)       # G_y fp16 AR payload
            gxp = sb.tile([P, GCOLS], F16)        # local Gx, scaled, x2 off-diag

            def gram_pass(src):
                for kk in range(KK):
                    for m in range(MS):
                        for (c0, c1) in _mm_chunks(WIDTHS[m]):
                            nc.tensor.matmul(
                                slabs[m][:, c0:c1],
                                src[:, 2 * kk:2 * kk + 2, P * m:P * (m + 1)],
                                src[:, 2 * kk:2 * kk + 2,
                                    P * m + c0:P * m + c1],
                                start=(kk == 0),
                                stop=(kk == KK - 1),
                                perf_mode=mybir.MatmulPerfMode.DoubleRow,
                                skip_group_check=True,
                            )

            gram_pass(yb)

            # ---- pack G_y slabs to fp16 (vector); both packs feed one
            # ReduceScatter. cin is laid out k-block-outer [8, 2, 128, 336]
            # so RS chunk c is exactly [2, 128, 336]: the (Gy, Gx) column
            # block pair c with the FULL 128 partitions - the readback is
            # two plain slices and the chunk dot keeps all DVE lanes ----
            CW = GCOLS // N_CORES                 # 336 cols per block
            cin = dram.tile([N_CORES, 2, P, CW], F16)
            cout = dram.tile([2, P, CW], F16)

            def stage_pack(src_sb, plane, k0, k1, eng):
                dst = cin[k0:k1, plane, :, :].rearrange("k p j -> p k j")
                src = src_sb[:, k0 * CW:k1 * CW].rearrange(
                    "p (k j) -> p k j", j=CW)
                eng.dma_start(dst, src)

            for m in range(MS):
                off, w = COFF[m], WIDTHS[m]
                nc.vector.tensor_copy(pack[:, off:off + w], slabs[m][:, 0:w])
            # 336-aligned staging pieces: cols 0:1344 ready after slab 1
            # (covers to col 1408), 1344:2688 after slab 5
            stage_pack(pack, 0, 0, 4, nc.gpsimd)
            stage_pack(pack, 0, 4, N_CORES, nc.gpsimd)

            # ---- G_x into the same PSUM slabs (WAR on the pack copies);
            # packed with the 2^-13 pre-scale and the x2 weight on
            # strict-right (off-diagonal) columns, staged next to G_y ----
            gram_pass(xb)
            for m in range(MS):
                off, w = COFF[m], WIDTHS[m]
                nc.scalar.mul(gxp[:, off:off + P], slabs[m][:, 0:P], SCALE)
                if w > P:
                    nc.scalar.mul(gxp[:, off + P:off + w],
                                  slabs[m][:, P:w], 2.0 * SCALE)
            stage_pack(gxp, 1, 0, 4, nc.sync)
            stage_pack(gxp, 1, 4, N_CORES, nc.gpsimd)

            nc.gpsimd.collective_compute(
                "ReduceScatter",
                mybir.AluOpType.add,
                replica_groups=[list(range(N_CORES))],
                ins=[cin.opt()],
                outs=[cout.opt()],
            )

            # ---- diagonal terms z_i = <x_i, y_i> from fp32 (vector) ----
            zscr = sb.tile([P, D], F32)
            zcols = sb.tile([P, KCH], F32)
            for k in range(KCH):
                nc.vector.scalar_tensor_tensor(
                    zscr[:], xt[:, k, :], 1.0, yt[:, k, :],
                    mybir.AluOpType.mult, mybir.AluOpType.mult,
                    accum_out=zcols[:, k:k + 1],
                )
            zsq = sb.tile([P, KCH], F32)
            stage = sb.tile([P, 3], F32)
            # stage cols 1-2 carry the z terms pre-divided by C0 (and
            # negated, since they're subtracted) so that C0 * sum_p(stage)
            # via the PE reduce yields the finished per-core contribution:
            #   col 1 -> -(2/N)*sum z,  col 2 -> -inv_nn1*sum z^2
            zred = sb.tile([P, 2], F32)
            nc.vector.tensor_reduce(zred[:, 0:1], zcols[:],
                                    mybir.AxisListType.X, mybir.AluOpType.add)
            nc.vector.scalar_tensor_tensor(
                zsq[:], zcols[:], 1.0, zcols[:],
                mybir.AluOpType.mult, mybir.AluOpType.mult,
                accum_out=zred[:, 1:2],
            )
            nc.vector.tensor_scalar_mul(stage[:, 1:2], zred[:, 0:1],
                                        -2.0 * SCALE * (N - 1))
            nc.vector.tensor_scalar_mul(stage[:, 2:3], zred[:, 1:2], -SCALE)

            # ---- after the ReduceScatter: read back this core's (Gy, Gx)
            # summed column-block pair, one fp16 STT dots them ----
            ga = sb.tile([P, CW], F16)
            gb = sb.tile([P, CW], F16)
            dscr = sb.tile([P, CW], F16)
            nc.sync.dma_start(ga[:], cout[0, :, :])
            nc.scalar.dma_start(gb[:], cout[1, :, :])
            nc.vector.scalar_tensor_tensor(
                dscr[:], ga[:], 1.0, gb[:],
                mybir.AluOpType.mult, mybir.AluOpType.mult,
                accum_out=stage[:, 0:1],
            )

            # ---- partition reduction via PE (wvec^T @ stage) applies the
            # C0 normalization; a single horizontal reduce then yields the
            # finished per-core loss contribution, so the scalar AllReduce
            # output IS the loss ----
            pfin = slabs[5][0:1, 0:3]
            nc.tensor.matmul(pfin, wvec[:, 0:1], stage[:, 0:3],
                             start=True, stop=True)
            lc = sb.tile([1, 1], F32)
            nc.vector.tensor_reduce(lc[:], pfin,
                                    mybir.AxisListType.X, mybir.AluOpType.add)

            cin2 = dram.tile([1, 1], F32)
            cout2 = dram.tile([1, 1], F32, addr_space="Shared")
            nc.sync.dma_start(cin2[:], lc[:])
            nc.gpsimd.collective_compute(
                "AllReduce",
                mybir.AluOpType.add,
                replica_groups=[list(range(N_CORES))],
                ins=[cin2.opt()],
                outs=[cout2.opt()],
            )
            nc.sync.dma_start(loss_ap[:], cout2[:])

    nc.compile()
    return nc


def _get_nc():
    if "nc" not in _CACHE:
        _CACHE["nc"] = _build()
    return _CACHE["nc"]


def _run(x, y, trace=False, **trace_kwargs):
    nc = _get_nc()
    x = np.ascontiguousarray(np.asarray(x, dtype=np.float32))
    y = np.ascontiguousarray(np.asarray(y, dtype=np.float32))
    assert x.shape == (N, D) and y.shape == (N, D)
    in_maps = [
        {"x": x[c * ROWS:(c + 1) * ROWS], "y": y[c * ROWS:(c + 1) * ROWS]}
        for c in range(N_CORES)
    ]
    res = run_bass_kernel_spmd(nc, in_maps, list(range(N_CORES)), trace=trace,
                               **trace_kwargs)
    loss = np.float32(res.results[0]["loss"][0, 0])
    return np.asarray(loss, dtype=np.float32).reshape(()), res


def kernel(x, y):
    out, _ = _run(x, y, trace=False)
    return out
